# revision 7
# baseline (speedup 1.0000x reference)
"""Trainium2 Bass kernel for nn_BlockDiagonalLinearAlignment.

Math: y = x @ A, where A is a 128x128 block-diagonal matrix assembled from
dense / diagonal / low-rank 16x16 blocks, followed by row-wise L2
normalization: out = y / (||y||_2 + 1e-8).

Strategy (pure data parallel over the batch axis, 8 cores):
  - host: assemble A once and hand each core a feature-major copy of its
    x shard (xT, [128, rows]). With features on partitions the PE can use
    x tiles directly as the stationary matmul operand -- no on-chip
    transpose, no PSUM->SBUF copy pass.
  - per 128-row tile: y_tile = lhsT(xT tile)^T @ A -> PSUM, batch-major.
  - per group of GROUP_TILES tiles: Act squares PSUM->SBUF, DVE reduces
    to ||y||^2, Act sqrt, DVE reciprocal, DVE broadcast-multiply into the
    output tile; chunked DMA out (rows interleaved across partitions).
  - engine budget per core: PE ~60us, Act ~45us, DVE ~85us, DMA ~104us
    (33.5 MB HBM roofline) -> DMA-bound.
"""

import contextlib
import functools
import sys

for _p in ("/opt/trn_rl_repo",):
    if _p not in sys.path:
        sys.path.append(_p)

import numpy as np

import concourse.bacc as bacc
import concourse.bass as bass
import concourse.tile as tile
from concourse import bass_utils, mybir

B = 262144
D = 128
BS = 16
K = 8
N_CORES = 8
ROWS_PER_CORE = B // N_CORES  # 32768

DENSE = (0, 3, 6)
DIAG = (1, 4, 7)
LR = (2, 5)

F32 = mybir.dt.float32
BF16 = mybir.dt.bfloat16

P = 128
CHUNK_ROWS = 2048  # rows per DMA chunk (per core)

# implementation variants (bisect/perf knobs)
MM_DTYPE = "f32"         # "f32" | "bf16": dtype of the xA matmul
SQ_DTYPE = "bf16"        # "f32" | "bf16": squared tile + n2 (bf16 = 2x DVE)
MUL_TILES_ON_ACT = 2     # tiles per group whose final scale runs on Act
GROUP_TILES = 8          # 128-row tiles per PSUM group
BUFS = dict(inpool=6, outpool=4, sqpool=4, smalls=12, psB=4)


def _assemble_A(W_dense, s_diag, U, V):
    """Full 128x128 block-diagonal transform, y = x @ A."""
    A = np.zeros((D, D), dtype=np.float32)
    for i, k in enumerate(DENSE):
        A[k * BS:(k + 1) * BS, k * BS:(k + 1) * BS] = W_dense[i].T
    for i, k in enumerate(DIAG):
        A[k * BS:(k + 1) * BS, k * BS:(k + 1) * BS] = np.diag(s_diag[i])
    for i, k in enumerate(LR):
        A[k * BS:(k + 1) * BS, k * BS:(k + 1) * BS] = V[i] @ U[i].T
    return A


def _kernel_body(ctx, tc, out_ap, xt_ap, amat_ap, rows, chunk_rows):
    nc = tc.nc
    tpc = chunk_rows // P          # 128-row tiles per chunk
    nchunks = rows // chunk_rows
    gt = GROUP_TILES
    ngroups = tpc // gt            # groups per chunk
    assert tpc % gt == 0 and rows % chunk_rows == 0

    xtv = xt_ap.rearrange("f (c n) -> c f n", c=nchunks)
    # host interleaves xT columns so tile j's partitions are rows p*tpc + j:
    # partition p of out_sb then holds rows [c*chunk + p*tpc, +tpc) contiguous
    ov = out_ap.rearrange("(c p r) f -> c p r f", c=nchunks, p=P)

    mmdt = BF16 if MM_DTYPE == "bf16" else F32
    sqdt = BF16 if SQ_DTYPE == "bf16" else F32

    consts = ctx.enter_context(tc.tile_pool(name="consts", bufs=1))
    amat_f32 = consts.tile([P, P], F32)
    nc.sync.dma_start(out=amat_f32, in_=amat_ap)
    if mmdt != F32:
        amat = consts.tile([P, P], mmdt)
        nc.scalar.copy(amat, amat_f32)
    else:
        amat = amat_f32

    inpool = ctx.enter_context(tc.tile_pool(name="inpool", bufs=BUFS["inpool"]))
    outpool = ctx.enter_context(tc.tile_pool(name="outpool", bufs=BUFS["outpool"]))
    sqpool = ctx.enter_context(tc.tile_pool(name="sqpool", bufs=BUFS["sqpool"]))
    smalls = ctx.enter_context(tc.tile_pool(name="smalls", bufs=BUFS["smalls"]))
    psB = ctx.enter_context(tc.tile_pool(name="psB", bufs=BUFS["psB"], space="PSUM"))

    for c in range(nchunks):
        xT_sb = inpool.tile([P, chunk_rows], mmdt)
        nc.sync.dma_start(out=xT_sb, in_=xtv[c])
        out_sb = outpool.tile([P, tpc, D], F32)

        for g in range(ngroups):
            y_ps = psB.tile([P, gt, D], F32)
            for j in range(gt):
                t = g * gt + j
                nc.tensor.matmul(
                    y_ps[:, j], lhsT=xT_sb[:, t * P:(t + 1) * P], rhs=amat,
                    start=True, stop=True,
                )

            sq = sqpool.tile([P, gt, D], sqdt)
            nc.scalar.activation(
                sq, y_ps, mybir.ActivationFunctionType.Square,
            )
            n2 = smalls.tile([P, gt], sqdt)
            if sqdt == F32:
                nc.vector.tensor_reduce(
                    n2, sq, axis=mybir.AxisListType.X, op=mybir.AluOpType.add,
                )
            else:
                with nc.allow_low_precision("norm^2 in bf16: rel err ~1e-3"):
                    nc.vector.tensor_reduce(
                        n2, sq, axis=mybir.AxisListType.X, op=mybir.AluOpType.add,
                    )
            nrm = smalls.tile([P, gt], F32)
            nc.scalar.sqrt(nrm, n2)
            rnorm = smalls.tile([P, gt], F32)
            nc.vector.reciprocal(rnorm, nrm)

            nd = gt - MUL_TILES_ON_ACT
            if nd:
                nc.vector.tensor_mul(
                    out_sb[:, g * gt:g * gt + nd, :],
                    y_ps[:, 0:nd],
                    rnorm[:, 0:nd].broadcast_to([P, nd, D]),
                )
            for j in range(nd, gt):
                nc.scalar.activation(
                    out_sb[:, g * gt + j, :], y_ps[:, j],
                    mybir.ActivationFunctionType.Copy,
                    scale=rnorm[:, j:j + 1],
                )

        # out-DMAs ride the Activation HWDGE queue so a compute-gated store
        # can't head-of-line-block the next chunk's load on the sync queue
        nc.scalar.dma_start(out=ov[c], in_=out_sb)


@functools.lru_cache(maxsize=4)
def _build(rows, chunk_rows):
    nc = bacc.Bacc(
        "TRN2",
        target_bir_lowering=False,
        debug=False,
        num_devices=1,
    )
    xt_t = nc.dram_tensor("xt", [D, rows], F32, kind="ExternalInput").ap()
    a_t = nc.dram_tensor("amat", [D, D], F32, kind="ExternalInput").ap()
    o_t = nc.dram_tensor("out", [rows, D], F32, kind="ExternalOutput").ap()
    with tile.TileContext(nc) as tc, contextlib.ExitStack() as ctx:
        _kernel_body(ctx, tc, o_t, xt_t, a_t, rows, chunk_rows)
    nc.compile()
    return nc


def _interleave_xt(xcore):
    """[rows, D] -> [D, rows] with columns ordered (chunk, j, p) where the
    original row index is c*CHUNK_ROWS + p*tpc + j (tpc = tiles per chunk).
    Tile j of a chunk then has row p*tpc+j on partition p, which makes the
    output tile layout land contiguous per partition for the store DMA."""
    nchunks = ROWS_PER_CORE // CHUNK_ROWS
    tpc = CHUNK_ROWS // P
    v = xcore.reshape(nchunks, P, tpc, D)
    return np.ascontiguousarray(v.transpose(3, 0, 2, 1)).reshape(D, ROWS_PER_CORE)


def _run(x, A, trace=False, trace_cores=None):
    nc = _build(ROWS_PER_CORE, CHUNK_ROWS)
    xs = x.reshape(N_CORES, ROWS_PER_CORE, D)
    in_maps = [
        {"xt": _interleave_xt(xs[i]), "amat": A}
        for i in range(N_CORES)
    ]
    res = bass_utils.run_bass_kernel_spmd(
        nc, in_maps, core_ids=list(range(N_CORES)),
        trace=trace, trace_cores=trace_cores,
    )
    out = np.concatenate([r["out"] for r in res.results], axis=0)
    return out, res


def kernel(x, W_dense, s_diag, U, V):
    A = _assemble_A(
        np.asarray(W_dense, dtype=np.float32),
        np.asarray(s_diag, dtype=np.float32),
        np.asarray(U, dtype=np.float32),
        np.asarray(V, dtype=np.float32),
    )
    out, _ = _run(np.asarray(x, dtype=np.float32), A)
    return out


# revision 8
# speedup vs baseline: 1.1771x; 1.1771x over previous
"""Trainium2 Bass kernel for nn_BlockDiagonalLinearAlignment.

Math: y = x @ A, where A is a 128x128 block-diagonal matrix assembled from
dense / diagonal / low-rank 16x16 blocks, followed by row-wise L2
normalization: out = y / (||y||_2 + 1e-8).

Strategy (pure data parallel over the batch axis, 8 cores):
  - host: assemble A once and hand each core a feature-major, tile-interleaved
    copy of its x shard (xT, [128, rows], bf16). With features on partitions
    the PE uses x tiles directly as the stationary matmul operand -- no
    on-chip transpose, no PSUM->SBUF copy pass. bf16 halves the input DMA.
  - the host interleave orders xT columns so tile t's partitions are rows
    p*tpc + t: the output tiles then land contiguous per partition and the
    store DMA gets 8KB lines.
  - per 128-row tile: y_tile = lhsT(xT tile)^T @ A -> PSUM f32, batch-major.
  - per group of GROUP_TILES tiles: Act squares PSUM->SBUF f32, DVE reduces
    to ||y||^2, Act sqrt, DVE reciprocal, DVE broadcast-multiply into an
    output tile, store DMA per group on the Act HWDGE queue (separate from
    the load queue on sync -> no head-of-line blocking).
"""

import contextlib
import functools
import sys

for _p in ("/opt/trn_rl_repo",):
    if _p not in sys.path:
        sys.path.append(_p)

import numpy as np

import concourse.bacc as bacc
import concourse.bass as bass
import concourse.tile as tile
from concourse import bass_utils, mybir

B = 262144
D = 128
BS = 16
K = 8
N_CORES = 8
ROWS_PER_CORE = B // N_CORES  # 32768

DENSE = (0, 3, 6)
DIAG = (1, 4, 7)
LR = (2, 5)

F32 = mybir.dt.float32
BF16 = mybir.dt.bfloat16

P = 128
CHUNK_ROWS = 4096  # rows per load-DMA chunk (per core)

# implementation variants (bisect/perf knobs)
MM_DTYPE = "bf16"        # "f32" | "bf16": dtype of xT in HBM + the xA matmul
GROUP_TILES = 16         # 128-row tiles per PSUM group (16 -> 2 PSUM bufs)
BUFS = dict(inpool=6, outpool=4, sqpool=4, smalls=12, psB=2)


def _assemble_A(W_dense, s_diag, U, V):
    """Full 128x128 block-diagonal transform, y = x @ A."""
    A = np.zeros((D, D), dtype=np.float32)
    for i, k in enumerate(DENSE):
        A[k * BS:(k + 1) * BS, k * BS:(k + 1) * BS] = W_dense[i].T
    for i, k in enumerate(DIAG):
        A[k * BS:(k + 1) * BS, k * BS:(k + 1) * BS] = np.diag(s_diag[i])
    for i, k in enumerate(LR):
        A[k * BS:(k + 1) * BS, k * BS:(k + 1) * BS] = V[i] @ U[i].T
    return A


def _kernel_body(ctx, tc, out_ap, xt_ap, amat_ap, rows, chunk_rows):
    nc = tc.nc
    tpc = chunk_rows // P          # 128-row tiles per chunk
    nchunks = rows // chunk_rows
    gt = GROUP_TILES
    ngroups = tpc // gt            # groups per chunk
    assert tpc % gt == 0 and rows % chunk_rows == 0

    mmdt = BF16 if MM_DTYPE == "bf16" else F32

    xtv = xt_ap.rearrange("f (c n) -> c f n", c=nchunks)
    # host interleaves xT columns so tile t's partitions are rows p*tpc + t:
    # partition p of group g's out tile holds rows g*gt..(g+1)*gt contiguous
    ov = out_ap.rearrange("(c p r) f -> c p r f", c=nchunks, p=P)

    consts = ctx.enter_context(tc.tile_pool(name="consts", bufs=1))
    amat_f32 = consts.tile([P, P], F32)
    nc.sync.dma_start(out=amat_f32, in_=amat_ap)
    if mmdt != F32:
        amat = consts.tile([P, P], mmdt)
        nc.scalar.copy(amat, amat_f32)
    else:
        amat = amat_f32

    inpool = ctx.enter_context(tc.tile_pool(name="inpool", bufs=BUFS["inpool"]))
    outpool = ctx.enter_context(tc.tile_pool(name="outpool", bufs=BUFS["outpool"]))
    sqpool = ctx.enter_context(tc.tile_pool(name="sqpool", bufs=BUFS["sqpool"]))
    smalls = ctx.enter_context(tc.tile_pool(name="smalls", bufs=BUFS["smalls"]))
    psB = ctx.enter_context(tc.tile_pool(name="psB", bufs=BUFS["psB"], space="PSUM"))

    for c in range(nchunks):
        xT_sb = inpool.tile([P, chunk_rows], mmdt)
        nc.sync.dma_start(out=xT_sb, in_=xtv[c])

        for g in range(ngroups):
            y_ps = psB.tile([P, gt, D], F32)
            for j in range(gt):
                t = g * gt + j
                nc.tensor.matmul(
                    y_ps[:, j], lhsT=xT_sb[:, t * P:(t + 1) * P], rhs=amat,
                    start=True, stop=True,
                )

            sq = sqpool.tile([P, gt, D], F32)
            nc.scalar.activation(
                sq, y_ps, mybir.ActivationFunctionType.Square,
            )
            n2 = smalls.tile([P, gt], F32)
            nc.vector.tensor_reduce(
                n2, sq, axis=mybir.AxisListType.X, op=mybir.AluOpType.add,
            )
            nrm = smalls.tile([P, gt], F32)
            nc.scalar.sqrt(nrm, n2)
            rnorm = smalls.tile([P, gt], F32)
            nc.vector.reciprocal(rnorm, nrm)

            og = outpool.tile([P, gt, D], F32)
            nc.vector.tensor_mul(
                og, y_ps, rnorm.broadcast_to([P, gt, D]),
            )
            # store per group on the Act HWDGE queue: a compute-gated store
            # can't head-of-line-block the next chunk's load on sync
            nc.scalar.dma_start(out=ov[c][:, g * gt:(g + 1) * gt, :], in_=og)


@functools.lru_cache(maxsize=4)
def _build(rows, chunk_rows):
    nc = bacc.Bacc(
        "TRN2",
        target_bir_lowering=False,
        debug=False,
        num_devices=1,
    )
    mmdt = BF16 if MM_DTYPE == "bf16" else F32
    xt_t = nc.dram_tensor("xt", [D, rows], mmdt, kind="ExternalInput").ap()
    a_t = nc.dram_tensor("amat", [D, D], F32, kind="ExternalInput").ap()
    o_t = nc.dram_tensor("out", [rows, D], F32, kind="ExternalOutput").ap()
    with tile.TileContext(nc) as tc, contextlib.ExitStack() as ctx:
        _kernel_body(ctx, tc, o_t, xt_t, a_t, rows, chunk_rows)
    nc.compile()
    return nc


def _interleave_xt(xcore):
    """[rows, D] -> [D, rows] with columns ordered (chunk, t, p) where the
    original row index is c*CHUNK_ROWS + p*tpc + t (tpc = tiles per chunk).
    Tile t of a chunk then has row p*tpc+t on partition p, which makes the
    output tile layout land contiguous per partition for the store DMA."""
    nchunks = ROWS_PER_CORE // CHUNK_ROWS
    tpc = CHUNK_ROWS // P
    v = xcore
    if MM_DTYPE == "bf16":
        import ml_dtypes
        v = v.astype(ml_dtypes.bfloat16)
    v = v.reshape(nchunks, P, tpc, D)
    return np.ascontiguousarray(v.transpose(3, 0, 2, 1)).reshape(D, ROWS_PER_CORE)


def _run(x, A, trace=False, trace_cores=None):
    nc = _build(ROWS_PER_CORE, CHUNK_ROWS)
    xs = x.reshape(N_CORES, ROWS_PER_CORE, D)
    in_maps = [
        {"xt": _interleave_xt(xs[i]), "amat": A}
        for i in range(N_CORES)
    ]
    res = bass_utils.run_bass_kernel_spmd(
        nc, in_maps, core_ids=list(range(N_CORES)),
        trace=trace, trace_cores=trace_cores,
    )
    out = np.concatenate([r["out"] for r in res.results], axis=0)
    return out, res


def kernel(x, W_dense, s_diag, U, V):
    A = _assemble_A(
        np.asarray(W_dense, dtype=np.float32),
        np.asarray(s_diag, dtype=np.float32),
        np.asarray(U, dtype=np.float32),
        np.asarray(V, dtype=np.float32),
    )
    out, _ = _run(np.asarray(x, dtype=np.float32), A)
    return out


# revision 11
# speedup vs baseline: 1.4116x; 1.1993x over previous
"""Trainium2 Bass kernel for nn_BlockDiagonalLinearAlignment.

Math: y = x @ A, where A is a 128x128 block-diagonal matrix assembled from
dense / diagonal / low-rank 16x16 blocks, followed by row-wise L2
normalization: out = y / (||y||_2 + 1e-8).

Strategy (pure data parallel over the batch axis, 8 cores):
  - host: assemble A once and hand each core a feature-major, tile-interleaved
    copy of its x shard (xT, [128, rows], bf16). With features on partitions
    the PE uses x tiles directly as the stationary matmul operand -- no
    on-chip transpose, no PSUM->SBUF copy pass. bf16 halves the input DMA.
  - the host interleave orders xT columns so tile t's partitions are rows
    p*tpc + t: the output tiles then land contiguous per partition and the
    store DMA gets 8KB lines.
  - per 128-row tile: y_tile = lhsT(xT tile)^T @ A -> PSUM f32, batch-major.
  - per group of GROUP_TILES tiles: Act squares PSUM->SBUF f32, DVE reduces
    to ||y||^2, Act sqrt, DVE reciprocal, DVE broadcast-multiply into an
    output tile, store DMA per group on the Act HWDGE queue (separate from
    the load queue on sync -> no head-of-line blocking).
"""

import contextlib
import functools
import sys

for _p in ("/opt/trn_rl_repo",):
    if _p not in sys.path:
        sys.path.append(_p)

import numpy as np

import concourse.bacc as bacc
import concourse.bass as bass
import concourse.tile as tile
from concourse import bass_utils, mybir

B = 262144
D = 128
BS = 16
K = 8
N_CORES = 8
ROWS_PER_CORE = B // N_CORES  # 32768

DENSE = (0, 3, 6)
DIAG = (1, 4, 7)
LR = (2, 5)

F32 = mybir.dt.float32
BF16 = mybir.dt.bfloat16

P = 128
CHUNK_ROWS = 4096  # rows per load-DMA chunk (per core)

# implementation variants (bisect/perf knobs)
MM_DTYPE = "bf16"        # "f32" | "bf16": dtype of xT in HBM + the xA matmul
GROUP_TILES = 8          # 128-row tiles per PSUM group (8 -> 4 PSUM bufs)
PAIR_GROUPS = 2          # y-groups sharing one sq/n2/store batch
BUFS = dict(inpool=6, outpool=4, sqpool=3, smalls=12, psB=4)


def _assemble_A(W_dense, s_diag, U, V):
    """Full 128x128 block-diagonal transform, y = x @ A."""
    A = np.zeros((D, D), dtype=np.float32)
    for i, k in enumerate(DENSE):
        A[k * BS:(k + 1) * BS, k * BS:(k + 1) * BS] = W_dense[i].T
    for i, k in enumerate(DIAG):
        A[k * BS:(k + 1) * BS, k * BS:(k + 1) * BS] = np.diag(s_diag[i])
    for i, k in enumerate(LR):
        A[k * BS:(k + 1) * BS, k * BS:(k + 1) * BS] = V[i] @ U[i].T
    return A


def _kernel_body(ctx, tc, out_ap, xt_ap, amat_ap, rows, chunk_rows):
    nc = tc.nc
    tpc = chunk_rows // P          # 128-row tiles per chunk
    nchunks = rows // chunk_rows
    gt = GROUP_TILES
    ngroups = tpc // gt            # groups per chunk
    assert tpc % gt == 0 and rows % chunk_rows == 0

    mmdt = BF16 if MM_DTYPE == "bf16" else F32

    xtv = xt_ap.rearrange("f (c n) -> c f n", c=nchunks)
    # host interleaves xT columns so tile t's partitions are rows p*tpc + t:
    # partition p of group g's out tile holds rows g*gt..(g+1)*gt contiguous
    ov = out_ap.rearrange("(c p r) f -> c p r f", c=nchunks, p=P)

    consts = ctx.enter_context(tc.tile_pool(name="consts", bufs=1))
    amat_f32 = consts.tile([P, P], F32)
    nc.sync.dma_start(out=amat_f32, in_=amat_ap)
    if mmdt != F32:
        amat = consts.tile([P, P], mmdt)
        nc.scalar.copy(amat, amat_f32)
    else:
        amat = amat_f32

    inpool = ctx.enter_context(tc.tile_pool(name="inpool", bufs=BUFS["inpool"]))
    outpool = ctx.enter_context(tc.tile_pool(name="outpool", bufs=BUFS["outpool"]))
    sqpool = ctx.enter_context(tc.tile_pool(name="sqpool", bufs=BUFS["sqpool"]))
    smalls = ctx.enter_context(tc.tile_pool(name="smalls", bufs=BUFS["smalls"]))
    psB = ctx.enter_context(tc.tile_pool(name="psB", bufs=BUFS["psB"], space="PSUM"))

    pg = PAIR_GROUPS
    bt = pg * gt                   # tiles per store batch
    assert ngroups % pg == 0
    for c in range(nchunks):
        xT_sb = inpool.tile([P, chunk_rows], mmdt)
        # loads ride the Act HWDGE queue: gated only on buffer-free, they
        # fire early; stores ride sync so neither stream blocks the other
        nc.scalar.dma_start(out=xT_sb, in_=xtv[c])

        for b in range(ngroups // pg):
            ys = []
            for g in range(pg):
                y_ps = psB.tile([P, gt, D], F32)
                for j in range(gt):
                    t = (b * pg + g) * gt + j
                    nc.tensor.matmul(
                        y_ps[:, j], lhsT=xT_sb[:, t * P:(t + 1) * P], rhs=amat,
                        start=True, stop=True,
                    )
                ys.append(y_ps)

            sq = sqpool.tile([P, bt, D], F32)
            for g in range(pg):
                nc.scalar.activation(
                    sq[:, g * gt:(g + 1) * gt, :], ys[g],
                    mybir.ActivationFunctionType.Square,
                )
            n2 = smalls.tile([P, bt], F32)
            nc.vector.tensor_reduce(
                n2, sq, axis=mybir.AxisListType.X, op=mybir.AluOpType.add,
            )
            nrm = smalls.tile([P, bt], F32)
            nc.scalar.sqrt(nrm, n2)
            rnorm = smalls.tile([P, bt], F32)
            nc.vector.reciprocal(rnorm, nrm)

            og = outpool.tile([P, bt, D], F32)
            for g in range(pg):
                nc.vector.tensor_mul(
                    og[:, g * gt:(g + 1) * gt, :], ys[g],
                    rnorm[:, g * gt:(g + 1) * gt].broadcast_to([P, gt, D]),
                )
            nc.sync.dma_start(
                out=ov[c][:, b * bt:(b + 1) * bt, :], in_=og,
            )


@functools.lru_cache(maxsize=4)
def _build(rows, chunk_rows):
    nc = bacc.Bacc(
        "TRN2",
        target_bir_lowering=False,
        debug=False,
        num_devices=1,
    )
    mmdt = BF16 if MM_DTYPE == "bf16" else F32
    xt_t = nc.dram_tensor("xt", [D, rows], mmdt, kind="ExternalInput").ap()
    a_t = nc.dram_tensor("amat", [D, D], F32, kind="ExternalInput").ap()
    o_t = nc.dram_tensor("out", [rows, D], F32, kind="ExternalOutput").ap()
    with tile.TileContext(nc) as tc, contextlib.ExitStack() as ctx:
        _kernel_body(ctx, tc, o_t, xt_t, a_t, rows, chunk_rows)
    nc.compile()
    return nc


def _interleave_xt(xcore):
    """[rows, D] -> [D, rows] with columns ordered (chunk, t, p) where the
    original row index is c*CHUNK_ROWS + p*tpc + t (tpc = tiles per chunk).
    Tile t of a chunk then has row p*tpc+t on partition p, which makes the
    output tile layout land contiguous per partition for the store DMA."""
    nchunks = ROWS_PER_CORE // CHUNK_ROWS
    tpc = CHUNK_ROWS // P
    v = xcore
    if MM_DTYPE == "bf16":
        import ml_dtypes
        v = v.astype(ml_dtypes.bfloat16)
    v = v.reshape(nchunks, P, tpc, D)
    return np.ascontiguousarray(v.transpose(3, 0, 2, 1)).reshape(D, ROWS_PER_CORE)


def _run(x, A, trace=False, trace_cores=None):
    nc = _build(ROWS_PER_CORE, CHUNK_ROWS)
    xs = x.reshape(N_CORES, ROWS_PER_CORE, D)
    in_maps = [
        {"xt": _interleave_xt(xs[i]), "amat": A}
        for i in range(N_CORES)
    ]
    res = bass_utils.run_bass_kernel_spmd(
        nc, in_maps, core_ids=list(range(N_CORES)),
        trace=trace, trace_cores=trace_cores,
    )
    out = np.concatenate([r["out"] for r in res.results], axis=0)
    return out, res


def kernel(x, W_dense, s_diag, U, V):
    A = _assemble_A(
        np.asarray(W_dense, dtype=np.float32),
        np.asarray(s_diag, dtype=np.float32),
        np.asarray(U, dtype=np.float32),
        np.asarray(V, dtype=np.float32),
    )
    out, _ = _run(np.asarray(x, dtype=np.float32), A)
    return out


# revision 13
# speedup vs baseline: 1.4198x; 1.0057x over previous
"""Trainium2 Bass kernel for nn_BlockDiagonalLinearAlignment.

Math: y = x @ A, where A is a 128x128 block-diagonal matrix assembled from
dense / diagonal / low-rank 16x16 blocks, followed by row-wise L2
normalization: out = y / (||y||_2 + 1e-8).

Strategy (pure data parallel over the batch axis, 8 cores):
  - host: assemble A once and hand each core a feature-major, tile-interleaved
    copy of its x shard (xT, [128, rows], bf16). With features on partitions
    the PE uses x tiles directly as the stationary matmul operand -- no
    on-chip transpose, no PSUM->SBUF copy pass. bf16 halves the input DMA.
  - the host interleave orders xT columns so tile t's partitions are rows
    p*tpc + t: the output tiles then land contiguous per partition and the
    store DMA gets 8KB lines.
  - per 128-row tile: y_tile = lhsT(xT tile)^T @ A -> PSUM f32, batch-major.
  - per group of GROUP_TILES tiles: Act squares PSUM->SBUF f32, DVE reduces
    to ||y||^2, Act sqrt, DVE reciprocal, DVE broadcast-multiply into an
    output tile, store DMA per group on the Act HWDGE queue (separate from
    the load queue on sync -> no head-of-line blocking).
"""

import contextlib
import functools
import sys

for _p in ("/opt/trn_rl_repo",):
    if _p not in sys.path:
        sys.path.append(_p)

import numpy as np

import concourse.bacc as bacc
import concourse.bass as bass
import concourse.tile as tile
from concourse import bass_utils, mybir

B = 262144
D = 128
BS = 16
K = 8
N_CORES = 8
ROWS_PER_CORE = B // N_CORES  # 32768

DENSE = (0, 3, 6)
DIAG = (1, 4, 7)
LR = (2, 5)

F32 = mybir.dt.float32
BF16 = mybir.dt.bfloat16
FP16 = mybir.dt.float16
SQDT = FP16

P = 128
CHUNK_ROWS = 4096  # rows per load-DMA chunk (per core)

# implementation variants (bisect/perf knobs)
MM_DTYPE = "bf16"        # "f32" | "bf16": dtype of xT in HBM + the xA matmul
GROUP_TILES = 8          # 128-row tiles per PSUM group (8 -> 4 PSUM bufs)
PAIR_GROUPS = 2          # y-groups sharing one sq/n2/store batch
BUFS = dict(inpool=6, outpool=4, sqpool=3, smalls=12, psB=4)


def _assemble_A(W_dense, s_diag, U, V):
    """Full 128x128 block-diagonal transform, y = x @ A."""
    A = np.zeros((D, D), dtype=np.float32)
    for i, k in enumerate(DENSE):
        A[k * BS:(k + 1) * BS, k * BS:(k + 1) * BS] = W_dense[i].T
    for i, k in enumerate(DIAG):
        A[k * BS:(k + 1) * BS, k * BS:(k + 1) * BS] = np.diag(s_diag[i])
    for i, k in enumerate(LR):
        A[k * BS:(k + 1) * BS, k * BS:(k + 1) * BS] = V[i] @ U[i].T
    return A


def _kernel_body(ctx, tc, out_ap, xt_ap, amat_ap, rows, chunk_rows):
    nc = tc.nc
    tpc = chunk_rows // P          # 128-row tiles per chunk
    nchunks = rows // chunk_rows
    gt = GROUP_TILES
    ngroups = tpc // gt            # groups per chunk
    assert tpc % gt == 0 and rows % chunk_rows == 0

    mmdt = BF16 if MM_DTYPE == "bf16" else F32

    xtv = xt_ap.rearrange("f (c n) -> c f n", c=nchunks)
    # host interleaves xT columns so tile t's partitions are rows p*tpc + t:
    # partition p of group g's out tile holds rows g*gt..(g+1)*gt contiguous
    ov = out_ap.rearrange("(c p r) f -> c p r f", c=nchunks, p=P)

    consts = ctx.enter_context(tc.tile_pool(name="consts", bufs=1))
    amat_f32 = consts.tile([P, P], F32)
    nc.sync.dma_start(out=amat_f32, in_=amat_ap)
    if mmdt != F32:
        amat = consts.tile([P, P], mmdt)
        nc.scalar.copy(amat, amat_f32)
    else:
        amat = amat_f32

    inpool = ctx.enter_context(tc.tile_pool(name="inpool", bufs=BUFS["inpool"]))
    outpool = ctx.enter_context(tc.tile_pool(name="outpool", bufs=BUFS["outpool"]))
    sqpool = ctx.enter_context(tc.tile_pool(name="sqpool", bufs=BUFS["sqpool"]))
    smalls = ctx.enter_context(tc.tile_pool(name="smalls", bufs=BUFS["smalls"]))
    psB = ctx.enter_context(tc.tile_pool(name="psB", bufs=BUFS["psB"], space="PSUM"))

    pg = PAIR_GROUPS
    bt = pg * gt                   # tiles per store batch
    assert ngroups % pg == 0

    def emit_muls(st):
        """Scale+store for a finished batch: DVE muls slot between the next
        batch's reduce and recip, hiding the Act sqrt round-trip."""
        c, b, ys, rnorm = st
        og = outpool.tile([P, bt, D], F32)
        for g in range(pg):
            nc.vector.tensor_mul(
                og[:, g * gt:(g + 1) * gt, :], ys[g],
                rnorm[:, g * gt:(g + 1) * gt].broadcast_to([P, gt, D]),
            )
        nc.sync.dma_start(out=ov[c][:, b * bt:(b + 1) * bt, :], in_=og)

    pending = None
    for c in range(nchunks):
        xT_sb = inpool.tile([P, chunk_rows], mmdt)
        # loads ride the Act HWDGE queue: gated only on buffer-free, they
        # fire early; stores ride sync so neither stream blocks the other
        nc.scalar.dma_start(out=xT_sb, in_=xtv[c])

        for b in range(ngroups // pg):
            ys = []
            for g in range(pg):
                y_ps = psB.tile([P, gt, D], F32)
                for j in range(gt):
                    t = (b * pg + g) * gt + j
                    nc.tensor.matmul(
                        y_ps[:, j], lhsT=xT_sb[:, t * P:(t + 1) * P], rhs=amat,
                        start=True, stop=True,
                    )
                ys.append(y_ps)

            sq = sqpool.tile([P, bt, D], SQDT)
            for g in range(pg):
                nc.scalar.activation(
                    sq[:, g * gt:(g + 1) * gt, :], ys[g],
                    mybir.ActivationFunctionType.Square,
                )
            n2 = smalls.tile([P, bt], SQDT)
            with nc.allow_low_precision("fp16 norm^2: ~5e-4 rel err"):
                nc.vector.tensor_reduce(
                    n2, sq, axis=mybir.AxisListType.X, op=mybir.AluOpType.add,
                )
            nrm = smalls.tile([P, bt], F32)
            nc.scalar.sqrt(nrm, n2)
            if pending is not None:
                emit_muls(pending)
            rnorm = smalls.tile([P, bt], F32)
            nc.vector.reciprocal(rnorm, nrm)
            pending = (c, b, ys, rnorm)
    emit_muls(pending)


@functools.lru_cache(maxsize=4)
def _build(rows, chunk_rows):
    nc = bacc.Bacc(
        "TRN2",
        target_bir_lowering=False,
        debug=False,
        num_devices=1,
    )
    mmdt = BF16 if MM_DTYPE == "bf16" else F32
    xt_t = nc.dram_tensor("xt", [D, rows], mmdt, kind="ExternalInput").ap()
    a_t = nc.dram_tensor("amat", [D, D], F32, kind="ExternalInput").ap()
    o_t = nc.dram_tensor("out", [rows, D], F32, kind="ExternalOutput").ap()
    with tile.TileContext(nc) as tc, contextlib.ExitStack() as ctx:
        _kernel_body(ctx, tc, o_t, xt_t, a_t, rows, chunk_rows)
    nc.compile()
    return nc


def _interleave_xt(xcore):
    """[rows, D] -> [D, rows] with columns ordered (chunk, t, p) where the
    original row index is c*CHUNK_ROWS + p*tpc + t (tpc = tiles per chunk).
    Tile t of a chunk then has row p*tpc+t on partition p, which makes the
    output tile layout land contiguous per partition for the store DMA."""
    nchunks = ROWS_PER_CORE // CHUNK_ROWS
    tpc = CHUNK_ROWS // P
    v = xcore
    if MM_DTYPE == "bf16":
        import ml_dtypes
        v = v.astype(ml_dtypes.bfloat16)
    v = v.reshape(nchunks, P, tpc, D)
    return np.ascontiguousarray(v.transpose(3, 0, 2, 1)).reshape(D, ROWS_PER_CORE)


def _run(x, A, trace=False, trace_cores=None):
    nc = _build(ROWS_PER_CORE, CHUNK_ROWS)
    xs = x.reshape(N_CORES, ROWS_PER_CORE, D)
    in_maps = [
        {"xt": _interleave_xt(xs[i]), "amat": A}
        for i in range(N_CORES)
    ]
    res = bass_utils.run_bass_kernel_spmd(
        nc, in_maps, core_ids=list(range(N_CORES)),
        trace=trace, trace_cores=trace_cores,
    )
    out = np.concatenate([r["out"] for r in res.results], axis=0)
    return out, res


def kernel(x, W_dense, s_diag, U, V):
    A = _assemble_A(
        np.asarray(W_dense, dtype=np.float32),
        np.asarray(s_diag, dtype=np.float32),
        np.asarray(U, dtype=np.float32),
        np.asarray(V, dtype=np.float32),
    )
    out, _ = _run(np.asarray(x, dtype=np.float32), A)
    return out
